# revision 44
# baseline (speedup 1.0000x reference)
"""KeepTopK kernel for Trainium2.

out[i, j] = x[i, j] if x[i, j] is among the top-8 of row i else 1e6.

Strategy (pure data parallel, 8 cores, 32768 rows each):
  per [128, 1024] block (512 rows, 4 rows per partition):
    DVE  : v8 = max8(x_seg)                  top-8 per 256-wide row
    DVE  : y  = match_replace(x, v8, BETA)   top-8 positions -> BETA
    ACT  : z  = -y + BETA                    0 at top-8, BETA - x else
    POOL : out = z + x                       exact x at top-8, BETA else
  input DMA on the SP HWDGE ring, output DMA on the ACT HWDGE ring, so
  input issue never queues behind a store whose compute isn't done.
  The store for block b is emitted on ACT two iterations later, when
  POOL(b) has already finished -- its embedded wait then resolves
  immediately instead of stalling ACT's FIFO (and the z activations
  behind it).  512-row blocks keep the per-block chain latency (and so
  pipeline ramp and drain) short; the ends taper to 256/128 rows.

Per-block engine busy vs the ~2.88us/block HBM roofline (1MB in+out
at ~360 GB/s): DVE 1.9us, POOL 2.4us, ACT 1.8us -- DMA-bound; the
measured steady-state cadence is 2.88us/block.  match_replace
replaces one occurrence per top-8 element in index order, matching
jax.lax.top_k tie semantics bitwise (z is exactly 0 at top-8, so
out = x exactly).
"""
import numpy as np
from contextlib import ExitStack

import concourse.bass as bass
import concourse.mybir as mybir
import concourse.tile as tile
from concourse.bass_utils import run_bass_kernel_spmd

N, E, K = 262144, 256, 8
BETA = 1000000.0
NCORES = 8
ROWS_PER_CORE = N // NCORES          # 32768
ROWS_PER_PART = 8                    # rows packed per SBUF partition
BLOCK_FREE = ROWS_PER_PART * E       # 2048
ROWS_PER_BLOCK = 128 * ROWS_PER_PART  # 1024
NBLOCKS = ROWS_PER_CORE // ROWS_PER_BLOCK  # 32

MAX_WAITS = 1


def split_sync_waits(nc, max_waits=MAX_WAITS):
    """walrus codegen rejects instructions with more than one embedded sync
    wait; hoist extras onto same-engine NoOps placed immediately before."""
    spill_id = 0
    for f in nc.m.functions:
        for bb in f.blocks:
            insts = list(bb.instructions)
            new_insts = []
            changed = False
            for inst in insts:
                si = inst.sync_info
                waits = list(si.on_wait) if si and si.on_wait else []
                if len(waits) > max_waits:
                    extra = waits[:-max_waits]
                    si.on_wait = waits[-max_waits:]
                    for j in range(0, len(extra), max_waits):
                        nop = mybir.InstNoOp(
                            name=f"waitspill-{spill_id}", ins=[], outs=[])
                        spill_id += 1
                        nop.engine = inst.engine
                        nop.sync_info = type(si)(
                            on_wait=extra[j:j + max_waits], on_update=[])
                        new_insts.append(nop)
                    changed = True
                new_insts.append(inst)
            if changed:
                bb.instructions = new_insts


def build():
    nc = bass.Bass("TRN2", target_bir_lowering=False, debug=False)
    x = nc.dram_tensor("x", [ROWS_PER_CORE, E], mybir.dt.float32,
                       kind="ExternalInput")
    out = nc.dram_tensor("out", [ROWS_PER_CORE, E], mybir.dt.float32,
                         kind="ExternalOutput")
    xap = x.ap()
    oap = out.ap()
    f32 = mybir.dt.float32
    with tile.TileContext(nc) as tc:
        with ExitStack() as ctx:
            xpool = ctx.enter_context(tc.tile_pool(name="x", bufs=8))
            ypool = ctx.enter_context(tc.tile_pool(name="y", bufs=4))
            zpool = ctx.enter_context(tc.tile_pool(name="z", bufs=4))
            opool = ctx.enter_context(tc.tile_pool(name="o", bufs=4))
            vpool = ctx.enter_context(tc.tile_pool(name="v8", bufs=6))
            r0 = 0
            pending = []  # delayed stores: [(dst, ot), ...]

            def compute_half(xt, cols0, rpp, zdst):
                v8 = vpool.tile([128, 8 * rpp], f32)
                yt = ypool.tile([128, rpp * E], f32)
                for s in range(rpp):
                    seg = slice(cols0 + s * E, cols0 + (s + 1) * E)
                    v = v8[:, s * 8:(s + 1) * 8]
                    nc.vector.max(v, xt[:, seg])
                    nc.vector.match_replace(yt[:, s * E:(s + 1) * E], v,
                                            xt[:, seg], BETA)
                nc.scalar.activation(zdst, yt[:],
                                     mybir.ActivationFunctionType.Copy,
                                     bias=BETA, scale=-1.0)

            def emit_single(rpp):
                nonlocal r0
                rows = 128 * rpp
                bfree = rpp * E
                src = xap[r0:r0 + rows, :].rearrange(
                    "(p r) e -> p (r e)", p=128)
                dst = oap[r0:r0 + rows, :].rearrange(
                    "(p r) e -> p (r e)", p=128)
                r0 += rows
                xt = xpool.tile([128, bfree], f32)
                nc.sync.dma_start(xt[:], src)
                zt = zpool.tile([128, bfree], f32)
                compute_half(xt, 0, rpp, zt[:])
                if len(pending) >= 2:
                    nc.scalar.dma_start(*pending.pop(0))
                ot = opool.tile([128, bfree], f32)
                nc.gpsimd.tensor_tensor(ot[:], zt[:], xt[:],
                                        op=mybir.AluOpType.add)
                pending.append((dst, ot[:]))

            def emit_pair():
                # two 512-row compute halves share one 1MB DMA each way:
                # the rpp-8 layout gives 8KB/partition descriptors (~4%
                # better HBM throughput than 4KB) while compute stages
                # keep the short 512-row chain
                nonlocal r0
                rows = 1024
                src = xap[r0:r0 + rows, :].rearrange(
                    "(p r) e -> p (r e)", p=128)
                dst = oap[r0:r0 + rows, :].rearrange(
                    "(p r) e -> p (r e)", p=128)
                r0 += rows
                xt = xpool.tile([128, 2048], f32)
                nc.sync.dma_start(xt[:, 0:1024], src[:, 0:1024])
                nc.sync.dma_start(xt[:, 1024:2048], src[:, 1024:2048])
                ot = opool.tile([128, 2048], f32)
                for half in range(2):
                    cols = slice(half * 1024, (half + 1) * 1024)
                    zt = zpool.tile([128, 1024], f32)
                    compute_half(xt, half * 1024, 4, zt[:])
                    if half == 0 and len(pending) >= 2:
                        nc.scalar.dma_start(*pending.pop(0))
                    nc.gpsimd.tensor_tensor(ot[:, cols], zt[:], xt[:, cols],
                                            op=mybir.AluOpType.add)
                pending.append((dst, ot[:]))

            for rpp in (2, 2):
                emit_single(rpp)
            for _ in range(31):
                emit_pair()
            for rpp in (2, 1, 1):
                emit_single(rpp)
            for p in pending:
                nc.scalar.dma_start(*p)
            assert r0 == ROWS_PER_CORE, r0
    split_sync_waits(nc)
    return nc


_nc_cache = None


def _get_nc():
    global _nc_cache
    if _nc_cache is None:
        _nc_cache = build()
    return _nc_cache


def kernel(x: np.ndarray, _trace: bool = False, **_trace_kwargs):
    x = np.ascontiguousarray(np.asarray(x, dtype=np.float32))
    assert x.shape == (N, E), x.shape
    nc = _get_nc()
    in_maps = [
        {"x": x[c * ROWS_PER_CORE:(c + 1) * ROWS_PER_CORE]}
        for c in range(NCORES)
    ]
    res = run_bass_kernel_spmd(nc, in_maps, core_ids=list(range(NCORES)),
                               trace=_trace, **_trace_kwargs)
    out = np.concatenate([res.results[c]["out"] for c in range(NCORES)],
                         axis=0)
    if _trace:
        return out, res
    return out
